# revision 32
# baseline (speedup 1.0000x reference)
"""Trainium2 Bass kernel for nn_Decoder_25486335935120.

Autoregressive LSTM decoder with categorical sampling, T=256 sequential steps.

Strategy (8 NeuronCores, model-parallel over the hidden/vocab dims):
  - jax.random.categorical(key, logits) == argmax(logits + gumbel(key, shape)),
    and the gumbel table is data-independent -> precomputed on host CPU.
  - After step 0, the fed-back input x is a one-hot token, so x @ Wi is a row
    gather of Wi (indirect DMA by the sampled token index).
  - Weights are column-sharded across the 8 cores and stay SBUF-resident:
    core k holds Wh[:, gate-cols k] [1024,512], Wp[:, vocab-slice k]
    [1024,1024], and gathers rows of Wi[:, gate-cols k] from HBM.
  - Matmuls run as a bf16 hi/lo split (h ~ hi+lo, W ~ Hi+Lo; accumulate
    hi@Hi + lo@Hi + hi@Lo in fp32 PSUM). This matches fp32 accuracy to
    ~3e-6 but streams at 1 cycle/row instead of fp32's 4.
  - Per step: AllGather of the h^T slice (bf16 hi|lo pair, [128,64]) ->
    full h^T; z/logits matmuls; local argmax over two 512-wide halves of
    the core's vocab slice (half 0 overlaps half 1's matmuls); AllGather
    of (max, global-idx) candidate pairs -> global argmax everywhere;
    indirect-DMA gather of Wi rows for the next step.
  - The LSTM cell runs on 32x128 fp32 tiles (gates packed [i|f|o|g] so one
    Sigmoid activation covers i,f,o).

Outputs: per-core (logits+gumbel) vocab slice [T,32,1024] + the token
sequence; host subtracts the gumbel table back out and reassembles full
logits [32,256,8192] and one-hot preds.
"""
import os
import sys

import numpy as np

for _p in ("/opt/trn_rl_repo", "/opt/trn_rl_repo/concourse"):
    if _p not in sys.path and os.path.isdir(_p):
        sys.path.append(_p)

import ml_dtypes
import concourse.bass as bass
import concourse.bacc as bacc
import concourse.mybir as mybir
import concourse.tile as tile
from concourse import bass_utils
from concourse.masks import make_identity

# Problem shapes (hardcoded per the contract).
B, T, V, H = 32, 256, 8192, 1024
NCORES = 8
VS = V // NCORES        # 1024 vocab columns per core
HS = H // NCORES        # 128 h columns per core
GS = 4 * H // NCORES    # 512 gate columns per core (128 per gate)
KT = H // 128           # 8 contraction tiles

f32 = mybir.dt.float32
bf16 = mybir.dt.bfloat16
u32 = mybir.dt.uint32
np_bf16 = ml_dtypes.bfloat16

_BUILD_CACHE: dict = {}
_GUMBEL_CACHE: dict = {}


def _gate_cols(k: int) -> np.ndarray:
    """Column indices of z/Wh/Wi (width 4H) owned by core k, packed [i|f|o|g].

    Gate order in the weights is flax (i, f, g, o); we pack [i, f, o, g] so a
    single Sigmoid activation covers the first 3*HS columns.
    """
    base = np.arange(HS) + k * HS
    return np.concatenate([base, H + base, 3 * H + base, 2 * H + base])


def build_bass(n_steps: int = T, g_len: int | None = None,
               no_collectives: bool = False, no_gather: bool = False,
               single_set: bool = False, no_argmax: bool = False):
    """Build + compile the SPMD Bass program (identical on all 8 cores).

    no_collectives / no_gather build timing-attribution variants that produce
    WRONG results (collectives replaced by local DRAM copies / the indirect
    gather replaced by a fixed-row DMA); only used by the local benchmarks.
    """
    if g_len is None:
        g_len = n_steps
    nc = bacc.Bacc("TRN2", target_bir_lowering=False, debug=False,
                   enable_asserts=False, num_devices=NCORES)

    wh_hi_in = nc.dram_tensor("wh_hi_in", [H, GS], bf16, kind="ExternalInput")
    wh_lo_in = nc.dram_tensor("wh_lo_in", [H, GS], bf16, kind="ExternalInput")
    wp_hi_in = nc.dram_tensor("wp_hi_in", [H, VS], bf16, kind="ExternalInput")
    wp_lo_in = nc.dram_tensor("wp_lo_in", [H, VS], bf16, kind="ExternalInput")
    wi_in = nc.dram_tensor("wi_in", [V, GS], f32, kind="ExternalInput")
    g_in = nc.dram_tensor("g_in", [g_len, B, VS], f32, kind="ExternalInput")
    zx0_in = nc.dram_tensor("zx0_in", [B, GS], f32, kind="ExternalInput")
    ht0_in = nc.dram_tensor("ht0_in", [H, 2 * B], bf16, kind="ExternalInput")
    c0_in = nc.dram_tensor("c0_in", [B, HS], f32, kind="ExternalInput")
    coff_in = nc.dram_tensor("coff_in", [B, 1], f32, kind="ExternalInput")

    logits_out = nc.dram_tensor("logits_out", [n_steps, B, VS], f32,
                                kind="ExternalOutput")
    tok_out = nc.dram_tensor("tok_out", [n_steps, B], u32, kind="ExternalOutput")

    rg = [list(range(NCORES))]
    NH = VS // 512  # psum column halves for logits

    with tile.TileContext(nc) as tc:
        with (
            tc.tile_pool(name="wpool", bufs=1) as wpool,
            tc.tile_pool(name="sb", bufs=2) as sb,
            tc.tile_pool(name="gpool", bufs=3) as gpool,
            tc.tile_pool(name="psum", bufs=2, space="PSUM") as ps,
            tc.tile_pool(name="dram", bufs=2, space="DRAM") as dr,
        ):
            # ---- one-time loads ----
            wh_hi = wpool.tile([128, KT, GS], bf16)
            nc.sync.dma_start(wh_hi[:], wh_hi_in[:].rearrange("(kt p) n -> p kt n", p=128))
            wh_lo = wpool.tile([128, KT, GS], bf16)
            nc.sync.dma_start(wh_lo[:], wh_lo_in[:].rearrange("(kt p) n -> p kt n", p=128))
            wp_hi = wpool.tile([128, KT, VS], bf16)
            nc.sync.dma_start(wp_hi[:], wp_hi_in[:].rearrange("(kt p) n -> p kt n", p=128))
            wp_lo = wpool.tile([128, KT, VS], bf16)
            nc.sync.dma_start(wp_lo[:], wp_lo_in[:].rearrange("(kt p) n -> p kt n", p=128))
            coff = wpool.tile([B, 1], f32)
            nc.sync.dma_start(coff[:], coff_in[:])
            ident = wpool.tile([B, B], f32)
            make_identity(nc, ident[:])

            c_prev = sb.tile([B, HS], f32, tag="c_state")
            nc.sync.dma_start(c_prev[:], c0_in[:])
            zx = sb.tile([B, GS], f32, tag="zx")
            nc.sync.dma_start(zx[:], zx0_in[:])
            ht_prev = sb.tile([128, KT, 2 * B], bf16, tag="ht_full")
            nc.sync.dma_start(ht_prev[:], ht0_in[:].rearrange("(kt p) b -> p kt b", p=128))

            def z_matmuls(ht_buf):
                """z = h @ Wh_slice via bf16 hi/lo 3-set accumulation."""
                z_ps = ps.tile([B, GS], f32, tag="z_ps")
                sets = [(slice(0, B), wh_hi), (slice(B, 2 * B), wh_hi),
                        (slice(0, B), wh_lo)]
                if single_set:
                    sets = sets[:1]
                ns = len(sets)
                for si, (hsl, w) in enumerate(sets):
                    for kt in range(KT):
                        nc.tensor.matmul(z_ps[:], lhsT=ht_buf[:, kt, hsl],
                                         rhs=w[:, kt, :],
                                         start=(si == 0 and kt == 0),
                                         stop=(si == ns - 1 and kt == KT - 1))
                return z_ps

            z_ps = z_matmuls(ht_prev)

            for t in range(n_steps):
                # ---- z = h@Wh + x@Wi(+bh), cell update ----
                z_sb = sb.tile([B, GS], f32, tag="z_sb")
                nc.vector.tensor_tensor(out=z_sb[:], in0=z_ps[:], in1=zx[:],
                                        op=mybir.AluOpType.add)
                sig = sb.tile([B, 3 * HS], f32, tag="sig")  # sigmoid(i|f|o)
                nc.scalar.activation(sig[:], z_sb[:, 0:3 * HS],
                                     mybir.ActivationFunctionType.Sigmoid)
                tg = sb.tile([B, HS], f32, tag="tg")        # tanh(g)
                nc.scalar.activation(tg[:], z_sb[:, 3 * HS:4 * HS],
                                     mybir.ActivationFunctionType.Tanh)
                fc = sb.tile([B, HS], f32, tag="fc")
                nc.vector.tensor_tensor(out=fc[:], in0=sig[:, HS:2 * HS], in1=c_prev[:],
                                        op=mybir.AluOpType.mult)
                ig = sb.tile([B, HS], f32, tag="ig")
                nc.vector.tensor_tensor(out=ig[:], in0=sig[:, 0:HS], in1=tg[:],
                                        op=mybir.AluOpType.mult)
                c_new = sb.tile([B, HS], f32, tag="c_state")
                nc.vector.tensor_tensor(out=c_new[:], in0=fc[:], in1=ig[:],
                                        op=mybir.AluOpType.add)
                tc_t = sb.tile([B, HS], f32, tag="tc")
                nc.scalar.activation(tc_t[:], c_new[:],
                                     mybir.ActivationFunctionType.Tanh)
                h_new = sb.tile([B, HS], f32, tag="h_new")
                nc.vector.tensor_tensor(out=h_new[:], in0=sig[:, 2 * HS:3 * HS],
                                        in1=tc_t[:], op=mybir.AluOpType.mult)

                # ---- AllGather h^T slice (bf16 hi|lo) -> full h^T ----
                ht_ps = ps.tile([HS, B], f32, tag="ht_ps")
                nc.tensor.transpose(ht_ps[:], h_new[:], ident[:])
                pack_h = sb.tile([HS, 2 * B], bf16, tag="pack_h")
                nc.vector.tensor_copy(pack_h[:, 0:B], ht_ps[:])
                nc.vector.tensor_tensor(out=pack_h[:, B:2 * B], in0=ht_ps[:],
                                        in1=pack_h[:, 0:B],
                                        op=mybir.AluOpType.subtract)
                agh_i = dr.tile([HS, 2 * B], bf16, tag="agh_i")
                nc.sync.dma_start(agh_i[:], pack_h[:])
                agh_o = dr.tile([NCORES, HS, 2 * B], bf16, tag="agh_o")
                if no_collectives:
                    nc.sync.dma_start(agh_o[0], agh_i[:])
                else:
                    nc.gpsimd.collective_compute(
                        "AllGather", mybir.AluOpType.bypass, replica_groups=rg,
                        ins=[agh_i[:]], outs=[agh_o[:]])
                ht_cur = sb.tile([128, KT, 2 * B], bf16, tag="ht_full")
                nc.sync.dma_start(ht_cur[:], agh_o[:].rearrange("k p b -> p k b"))

                # ---- logits halves = h @ Wp_slice (bf16 hi/lo 3-set) ----
                lg_ps = ps.tile([B, VS], f32, tag="lg_ps")
                for nh in range(NH):
                    nsl = slice(nh * 512, (nh + 1) * 512)
                    sets = [(slice(0, B), wp_hi), (slice(B, 2 * B), wp_hi),
                            (slice(0, B), wp_lo)]
                    if single_set:
                        sets = sets[:1]
                    ns = len(sets)
                    for si, (hsl, w) in enumerate(sets):
                        for kt in range(KT):
                            nc.tensor.matmul(
                                lg_ps[:, nsl], lhsT=ht_cur[:, kt, hsl],
                                rhs=w[:, kt, nsl],
                                start=(si == 0 and kt == 0),
                                stop=(si == ns - 1 and kt == KT - 1))

                # ---- logits+gumbel and per-half local argmax ----
                # (half 0's vector work overlaps half 1's matmuls)
                g_t = gpool.tile([B, VS], f32, tag="g_t")
                nc.sync.dma_start(g_t[:], g_in[t])
                lgg = sb.tile([B, VS], f32, tag="lgg")
                nc.vector.tensor_tensor(out=lgg[:], in0=lg_ps[:], in1=g_t[:],
                                        op=mybir.AluOpType.add)
                pack = sb.tile([B, 2], f32, tag="pack")
                if no_argmax:
                    nc.vector.memset(pack[:], 1.0)
                else:
                    mx8 = sb.tile([B, 8], f32, tag="mx8")
                    nc.vector.max(mx8[:], lgg[:])
                    ix8 = sb.tile([B, 8], u32, tag="ix8")
                    nc.vector.max_index(ix8[:], mx8[:], lgg[:])
                    # global idx = local idx + core offset (u32 in, f32 out)
                    nc.vector.tensor_scalar(out=pack[:, 1:2], in0=ix8[:, 0:1],
                                            scalar1=coff[:], scalar2=None,
                                            op0=mybir.AluOpType.add)
                    nc.vector.tensor_copy(pack[:, 0:1], mx8[:, 0:1])
                nc.sync.dma_start(logits_out[t], lgg[:])

                # ---- global argmax via candidate AllGather ----
                cand_i = dr.tile([B, 2], f32, tag="cand_i")
                nc.sync.dma_start(cand_i[:], pack[:])
                cand_o = dr.tile([NCORES, B, 2], f32, tag="cand_o")
                if no_collectives:
                    nc.sync.dma_start(cand_o[0], cand_i[:])
                else:
                    nc.gpsimd.collective_compute(
                        "AllGather", mybir.AluOpType.bypass, replica_groups=rg,
                        ins=[cand_i[:]], outs=[cand_o[:]])
                cand = sb.tile([B, NCORES, 2], f32, tag="cand")
                nc.sync.dma_start(cand[:], cand_o[:].rearrange("k b j -> b k j"))

                m1 = sb.tile([B, 1], f32, tag="m1")
                nc.vector.reduce_max(m1[:], cand[:, :, 0], axis=mybir.AxisListType.X)
                msk = sb.tile([B, NCORES], f32, tag="msk")
                nc.vector.tensor_scalar(out=msk[:], in0=cand[:, :, 0], scalar1=m1[:],
                                        scalar2=None, op0=mybir.AluOpType.is_equal)
                wgt = sb.tile([B, NCORES], f32, tag="wgt")
                nc.vector.tensor_tensor(out=wgt[:], in0=msk[:], in1=cand[:, :, 1],
                                        op=mybir.AluOpType.mult)
                toku = sb.tile([B, 1], u32, tag="toku")
                with nc.allow_low_precision(reason="one-hot sum of small ints is exact"):
                    nc.vector.reduce_sum(toku[:], wgt[:], axis=mybir.AxisListType.X)
                nc.sync.dma_start(tok_out[t, :, None], toku[:])

                if t + 1 < n_steps:
                    # ---- gather Wi rows for the next step's x @ Wi ----
                    zx = sb.tile([B, GS], f32, tag="zx")
                    if no_gather:
                        nc.sync.dma_start(zx[:], wi_in[t:t + B])
                    else:
                        nc.gpsimd.indirect_dma_start(
                            out=zx[:], out_offset=None, in_=wi_in[:],
                            in_offset=bass.IndirectOffsetOnAxis(ap=toku[:, :1], axis=0))
                    # ---- z(t+1) = h(t) @ Wh (PE idle during argmax/gather) ----
                    z_ps = z_matmuls(ht_cur)
                c_prev = c_new

    nc.compile()
    return nc


def _gumbel_slices(n_steps: int):
    """Per-core gumbel tables [n_steps, B, VS], computed on host CPU.

    Must reproduce jax.random.categorical's internal noise exactly:
    key(42) split per scan step, gumbel(key, (B, V), float32).
    """
    import jax
    import jax.numpy as jnp
    if n_steps in _GUMBEL_CACHE:
        return _GUMBEL_CACHE[n_steps]
    cpu = jax.devices("cpu")[0]
    with jax.default_device(cpu):
        rng = jax.random.key(42)
        keys = []
        for _ in range(n_steps):
            rng, cat = jax.random.split(rng)
            keys.append(cat)
        # NB: must be a per-key loop — vmap over keys yields different bits
        # than the sequential categorical calls inside the reference scan.
        gf = jax.jit(lambda k: jax.random.gumbel(k, (B, V), jnp.float32))
        G = np.stack([np.asarray(gf(k)) for k in keys])  # [n_steps, B, V]
    out = [np.ascontiguousarray(G[:, :, k * VS:(k + 1) * VS]) for k in range(NCORES)]
    _GUMBEL_CACHE[n_steps] = out
    return out


def _bf16_split(w: np.ndarray):
    hi = w.astype(np_bf16)
    lo = (w - hi.astype(np.float32)).astype(np_bf16)
    return hi, lo


def prepare_in_maps(inputs, Wi, Wh, bh, Wp, bp, c0, h0, n_steps: int = T,
                    g_len: int | None = None):
    if g_len is None:
        g_len = n_steps
    Wi = np.asarray(Wi, np.float32)
    Wh = np.asarray(Wh, np.float32)
    bh = np.asarray(bh, np.float32)
    Wp = np.asarray(Wp, np.float32)
    bp = np.asarray(bp, np.float32)
    x0 = np.asarray(inputs[:, 0, :], np.float32)
    zx0 = x0 @ Wi + bh  # step-0 dense input contribution [B, 4H]
    # h0 transposed, bf16 hi|lo packed [H, 2B]
    ht0 = np.ascontiguousarray(np.asarray(h0, np.float32).T)  # [H, B]
    ht0_hi, ht0_lo = _bf16_split(ht0)
    ht0_pack = np.concatenate([ht0_hi, ht0_lo], axis=1)  # [H, 2B] bf16
    g_slices = _gumbel_slices(g_len)
    wi_b = Wi + bh[None, :]  # fold bh into the gathered rows (bh is zeros)
    in_maps = []
    g_raw = []
    for k in range(NCORES):
        g_raw.append(g_slices[k])
        cols = _gate_cols(k)
        wh_hi, wh_lo = _bf16_split(np.ascontiguousarray(Wh[:, cols]))
        wp_hi, wp_lo = _bf16_split(np.ascontiguousarray(Wp[:, k * VS:(k + 1) * VS]))
        coff = np.full((B, 1), k * VS, np.float32)
        in_maps.append({
            "wh_hi_in": wh_hi, "wh_lo_in": wh_lo,
            "wp_hi_in": wp_hi, "wp_lo_in": wp_lo,
            "wi_in": np.ascontiguousarray(wi_b[:, cols]),
            "g_in": g_slices[k] + bp[None, None, k * VS:(k + 1) * VS],
            "zx0_in": np.ascontiguousarray(zx0[:, cols]),
            "ht0_in": ht0_pack,
            "c0_in": np.ascontiguousarray(np.asarray(c0, np.float32)[:, k * HS:(k + 1) * HS]),
            "coff_in": coff,
        })
    return in_maps, g_raw


def assemble_outputs(results, g_raw, n_steps: int = T):
    logits = np.empty((B, n_steps, V), np.float32)
    for k in range(NCORES):
        lgg = results[k]["logits_out"] - g_raw[k][:n_steps]  # remove the gumbel
        logits[:, :, k * VS:(k + 1) * VS] = lgg.transpose(1, 0, 2)
    tok = results[0]["tok_out"].astype(np.int64)  # [n_steps, B]
    preds = np.zeros((B, n_steps, V), np.float32)
    bi = np.arange(B)[:, None]
    ti = np.arange(n_steps)[None, :]
    preds[bi, ti, tok.T] = 1.0
    return logits, preds, tok.T  # tok as [B, n_steps]


def run(inputs, Wi, Wh, bh, Wp, bp, c0, h0, n_steps: int = T, trace: bool = False):
    if n_steps not in _BUILD_CACHE:
        _BUILD_CACHE[n_steps] = build_bass(n_steps)
    nc = _BUILD_CACHE[n_steps]
    in_maps, g_raw = prepare_in_maps(inputs, Wi, Wh, bh, Wp, bp, c0, h0, n_steps)
    try:
        res = bass_utils.run_bass_kernel_spmd(
            nc, in_maps, core_ids=list(range(NCORES)), trace=trace)
    except Exception:
        # transient NRT execute errors have been observed; retry once
        res = bass_utils.run_bass_kernel_spmd(
            nc, in_maps, core_ids=list(range(NCORES)), trace=trace)
    logits, preds, tok = assemble_outputs(res.results, g_raw, n_steps)
    return logits, preds, tok, res


def kernel(inputs, Wi, Wh, bh, Wp, bp, c0, h0):
    logits, preds, _tok, _res = run(inputs, Wi, Wh, bh, Wp, bp, c0, h0)
    return logits, preds
